# revision 39
# baseline (speedup 1.0000x reference)
"""Distributed 2-layer GCN on 8 Trainium2 NeuronCores (Bass/Tile).

Math (reference, with norm='both' GraphConv and edge weights):
    h1   = (feat * do^-1/2) @ W1
    agg1 = segsum(h1[src] * ew, dst);  out1 = relu(agg1 * di^-1/2 + b1)
    h2   = (out1 * do^-1/2) @ W2
    agg2 = segsum(h2[src] * ew, dst);  out  = agg2 * di^-1/2 + b2

All diagonal scalings commute with the linear maps, so with
    wt_e = ew_e * do[src_e]^-1/2 * di[dst_e]^-1/2     (host-precomputed)
both layers reduce to the same sparse op Y = A~ @ X:
    X1   = feat @ W1
    out1 = relu(A~ @ X1 + b1)            (b1 == 0 in this problem)
    out  = (A~ @ (out1 @ W2)) + b2       (b2 added on host)

Per-core plan (nodes sharded 6250/core, edges partitioned by dst shard):
  stage0 : X1_mine = featT_shard.T @ W1pad          (PE, bf16)
  AG-1   : all-gather X1 -> X1_full [50000,256] bf16 in each core's HBM
  SpMM-1 : dma_gather rows X1_full[src] (512B rows) -> one-hot matmuls
           (host-built bf16 [128x64] tiles with wt folded in) accumulate
           dst-windows of 64 ranks in PSUM -> relu -> X2_mine
  Y2     : fused per 128-row block into SpMM-1's window drains
           (block DMA -> DMA-transpose -> @W2pad), hiding the whole
           phase under the layer-1 gather tail
  AG-2   : all-gather Y2 -> Y2_full [50000,128] bf16
  SpMM-2 : same indices/one-hots against Y2_full (256B rows) -> out windows
  out    : [6272, 64] f32 per core; host concatenates [:6250, :20] + b2

int16 gather-index limit (32767) handled by splitting each window's edges
into a "lo" stream (table row < 31250, i.e. src owned by cores 0-4) and a
"hi" stream (row >= 31250).  The op schedule (windows x max-over-cores op
counts) is identical on all 8 cores (SPMD); only the index/one-hot payloads
differ per core.

Perf notes (measured on this terminal):
  - Gather phases are bound by SWDGE ring descriptor processing
    (~8-10 ns/desc/ring, mostly independent of row bytes), not by Q7
    desc-gen (~4.6 ns/desc).  Two SWDGE queues exist (NRT programs only
    2; 4 crashes the device) with desc-gen on Q7 core pairs (2q, 2q+1)
    and independent rings -- every gather is split in half across both
    queues, halving the gather wall-clock.
  - single_packet=True can never work for big gathers (a packet must fit
    the 16KB xbar buffer) -- always pass single_packet=False.
  - AllGather outputs are addr_space="Shared" Internal tensors (the NRT
    HBM-HBM fast path).  "Shared" scratchpad is only PAIR-shared
    (cores 0-1, 2-3, ...), so direct SPMD writes cannot replace the
    collectives.  Strided collective outputs crash at runtime.
  - Nodes are permuted host-side: cores balanced by in-degree (snake
    deal), then each core's nodes packed into 64-rank windows targeting
    shared multiple-of-128 edge quotas per (window, stream), which cuts
    gather padding from ~16% to ~3%.
  - Y2 @W2 matmuls are deferred one 128-row block so the in-order PE
    never head-of-line blocks on a DMA transpose in flight; L2 output
    blocks stream to HBM as their windows drain.
  - fp8 X1 table (default on): ~80us faster (AG-1 halves + L1 ring time
    drops) at rel-err 1.25e-2 vs bf16's 2.5e-3 (gate 2e-2, deterministic
    seeded inputs).  GCN_FP8=0 reverts to bf16.
"""

import os

import numpy as np
import ml_dtypes

N_QUEUES = int(os.environ.get("GCN_NQ", "2"))  # SWDGE queues (desc-gen core pairs)
SINGLE_PACKET = os.environ.get("GCN_SP", "0") == "1"
FP8_L1 = os.environ.get("GCN_FP8", "1") == "1"  # fp8 X1 table: ~80us faster, rel-err 1.25e-2 (gate 2e-2)

N_NODES = 50000
N_EDGES = 800000
NCORES = 8
SHARD = N_NODES // NCORES          # 6250
SHARD_PAD = 6272                   # 49 * 128
NRT = SHARD_PAD // 128             # 49 row tiles
D_IN = 768
D_H = 256                          # hidden padded 200 -> 256 (512B bf16 rows)
D_Y2 = 128                         # layer-2 table cols, 20 valid (256B bf16 rows)
D_O = 64                           # output cols padded 20 -> 64
WIN = 64                           # dst ranks per PSUM window
NWIN = SHARD_PAD // WIN            # 98
# Sub-block-major layout for pipelined AllGathers (GCN_SUBAG=1) measured
# net-worse: the rank-based lo/hi pool packing costs ~9% gather padding,
# which at the ~8.7ns/desc ring floor outweighs the ~90us of AG overlap.
SUBAG = os.environ.get("GCN_SUBAG", "0") == "1"
SUB_BOUNDS = [0, 1664, 3200, 4736, SHARD]
LO_ROWS = 8 * 3200 if SUBAG else 5 * (N_NODES // 8)
HI_ROWS = N_NODES - LO_ROWS
# gather chunks as window ranges: 13x7 + 4 + 3.  The short final chunks
# shrink the serial compute tail between the last gather and AG-2 (and
# after L2's last gather), where the rings sit idle anyway.
CHUNK_BOUNDS = list(range(0, 92, 7)) + [95, NWIN]
CHUNKS = list(zip(CHUNK_BOUNDS[:-1], CHUNK_BOUNDS[1:]))
NCHUNK = len(CHUNKS)

nbf16 = ml_dtypes.bfloat16


# ----------------------------------------------------------------------------
# Host-side schedule construction
# ----------------------------------------------------------------------------

def _pack_idx(idx: np.ndarray) -> np.ndarray:
    """[n] -> [128, n/16] int16 wrap-16 + replicate-8 SBUF layout."""
    n = idx.shape[0]
    assert n % 16 == 0
    wrapped = idx.reshape(n // 16, 16).T.astype(np.int16)
    return np.tile(wrapped, (8, 1))


def _build_host_data(src, dst, edge_w):
    src = np.asarray(src).astype(np.int64)
    dst = np.asarray(dst).astype(np.int64)
    ew = np.asarray(edge_w).astype(np.float64)

    deg_out = np.bincount(src, minlength=N_NODES).clip(1).astype(np.float64)
    deg_in = np.bincount(dst, minlength=N_NODES).clip(1).astype(np.float64)
    wt = (ew * (deg_out[src] ** -0.5) * (deg_in[dst] ** -0.5)).astype(np.float32)

    # ---- node -> core assignment (balance total in-degree, snake deal) ----
    din = np.bincount(dst, minlength=N_NODES)
    order = np.argsort(-din, kind="stable")
    core_of = np.empty(N_NODES, np.int64)
    blocks = order.reshape(SHARD, NCORES)
    for r in range(SHARD):
        cs = range(NCORES) if r % 2 == 0 else range(NCORES - 1, -1, -1)
        for j, c in enumerate(cs):
            core_of[blocks[r, j]] = c

    margin = 12 * NWIN

    def quotas(total):
        s = int(np.ceil((total + margin) / 128))
        base, extra = s // NWIN, s % NWIN
        q = np.full(NWIN, base, np.int64)
        q[:extra] += 1
        return q * 128

    caps = np.full(NWIN, 64)
    caps[NWIN - 1] = SHARD - 64 * (NWIN - 1)  # last window short (42)

    # lo/hi stream designation.  SUBAG: lo = ranks < B0 (windows 0..WLO-1),
    # designated per-core before packing (fixes the rank/stream circularity
    # at the cost of pool-restricted packing).  Default: lo = src cores 0-4
    # (free single-pool packing, ~3% padding).
    if SUBAG:
        B0 = LO_ROWS // 8
        slot = np.arange(SHARD)
        is_lo_slot = ((slot + 1) * B0) // SHARD > (slot * B0) // SHARD
        lo_node = np.zeros(N_NODES, bool)
        for m in range(NCORES):
            nodes = np.where(core_of == m)[0]
            nodes = nodes[np.argsort(-din[nodes], kind="stable")]
            lo_node[nodes[is_lo_slot]] = True
        WLO = B0 // WIN
    else:
        lo_node = core_of < (LO_ROWS // SHARD)
        WLO = None

    d_lo = np.bincount(dst[lo_node[src]], minlength=N_NODES)
    d_hi = np.bincount(dst[~lo_node[src]], minlength=N_NODES)
    elo_max = max((d_lo[core_of == m]).sum() for m in range(NCORES))
    ehi_max = max((d_hi[core_of == m]).sum() for m in range(NCORES))
    q_lo, q_hi = quotas(elo_max), quotas(ehi_max)
    rank_of = np.empty(N_NODES, np.int64)  # dense rank within core
    for m in range(NCORES):
        win_nodes = [[] for _ in range(NWIN)]
        cnodes = np.where(core_of == m)[0]
        if SUBAG:
            pools = ((cnodes[lo_node[cnodes]], 0, WLO),
                     (cnodes[~lo_node[cnodes]], WLO, NWIN))
        else:
            pools = ((cnodes, 0, NWIN),)
        for pool_nodes, w0, w1 in pools:
            w_l = d_lo[pool_nodes].astype(np.float64)
            w_h = d_hi[pool_nodes].astype(np.float64)
            order_p = np.argsort(-(w_l + w_h), kind="stable")
            pool_nodes = pool_nodes[order_p]
            w_l, w_h = w_l[order_p], w_h[order_p]
            rem_lo = q_lo[w0:w1].astype(np.float64).copy()
            rem_hi = q_hi[w0:w1].astype(np.float64).copy()
            rem_cap = caps[w0:w1].astype(np.float64).copy()
            for i in range(pool_nodes.shape[0]):
                score = np.minimum(rem_lo - w_l[i], rem_hi - w_h[i])
                score[rem_cap <= 0] = -1e18
                w = int(np.argmax(score))
                win_nodes[w0 + w].append(pool_nodes[i])
                rem_lo[w] -= w_l[i]
                rem_hi[w] -= w_h[i]
                rem_cap[w] -= 1
        r = 0
        for w in range(NWIN):
            for u in win_nodes[w]:
                rank_of[u] = r
                r += 1
        assert r == SHARD

    if SUBAG:
        sb = np.array(SUB_BOUNDS, np.int64)
        k_of = np.searchsorted(sb, rank_of, side="right") - 1
        pos_of = 8 * sb[k_of] + core_of * (sb[k_of + 1] - sb[k_of]) \
            + (rank_of - sb[k_of])
    else:
        pos_of = core_of * SHARD + rank_of

    # ---- per-core per-stream edge lists sorted by window ----
    cores = []
    for m in range(NCORES):
        sel = core_of[dst] == m
        es, rk, w_ = pos_of[src[sel]], rank_of[dst[sel]], wt[sel]
        s_lo = es < LO_ROWS
        streams = {}
        for sname, mask in (("lo", s_lo), ("hi", ~s_lo)):
            e_, r_, ww = es[mask], rk[mask], w_[mask]
            o = np.argsort(r_ // WIN, kind="stable")
            e_, r_, ww = e_[o], r_[o], ww[o]
            cnt = np.bincount(r_ // WIN, minlength=NWIN)
            streams[sname] = (e_, r_, ww, cnt)
        cores.append(streams)

    K = {}
    for sname in ("lo", "hi"):
        counts = np.stack([cores[m][sname][3] for m in range(NCORES)], 0)
        k = np.ceil(counts.max(0) / 128).astype(np.int64)
        if sname == "lo":
            k = np.maximum(k, 1)
        K[sname] = k

    chunk_slots = {s: [int(K[s][w0:w1].sum()) for (w0, w1) in CHUNKS]
                   for s in ("lo", "hi")}
    nops = int(K["lo"].sum() + K["hi"].sum())

    # ---- per-core payloads ----
    payloads = []
    for m in range(NCORES):
        oh = np.zeros((nops, 128, WIN), np.float32)
        idx_arr = {"lo": [], "hi": []}
        opi = 0
        off = {"lo": 0, "hi": 0}
        for w in range(NWIN):
            for sname in ("lo", "hi"):
                e_, r_, ww, cnt = cores[m][sname]
                n_w = int(cnt[w])
                ebase = off[sname]
                rebase = LO_ROWS if sname == "hi" else 0
                nrows_s = HI_ROWS if sname == "hi" else LO_ROWS
                for j in range(int(K[sname][w])):
                    lo_ = min(j * 128, n_w)
                    hi_ = min((j + 1) * 128, n_w)
                    nreal = hi_ - lo_
                    # spread pad reads over the table to avoid hot rows
                    sl = ((opi * 131 + np.arange(128) * 37) % nrows_s)
                    if nreal > 0:
                        eidx = slice(ebase + lo_, ebase + hi_)
                        sl[:nreal] = e_[eidx] - rebase
                        oh[opi, np.arange(nreal), r_[eidx] - w * WIN] = ww[eidx]
                    idx_arr[sname].append(sl)
                    opi += 1
                off[sname] += n_w
        assert opi == nops
        oh_b = np.ascontiguousarray(
            np.transpose(oh.astype(nbf16), (1, 0, 2)).reshape(128, nops * WIN))
        pay = {"oh": oh_b}
        for sname in ("lo", "hi"):
            flat = (np.concatenate(idx_arr[sname]) if idx_arr[sname]
                    else np.zeros(0, np.int64))
            pay["idx_" + sname] = _pack_idx(flat)
        payloads.append(pay)

    sched = {
        "K": K,
        "chunk_slots": chunk_slots,
        "nops": nops,
        "core_of": core_of,
        "rank_of": rank_of,
        "pos_of": pos_of,
    }
    return sched, payloads


# ----------------------------------------------------------------------------
# Bass program
# ----------------------------------------------------------------------------

def _build_program(sched, with_b1=False, stage=99):
    import concourse.bacc as bacc
    import concourse.tile as tile
    import concourse.mybir as mybir

    BF16, F32, I16 = mybir.dt.bfloat16, mybir.dt.float32, mybir.dt.int16
    X1DT = mybir.dt.float8e4 if FP8_L1 else BF16
    K = sched["K"]
    chunk_slots = sched["chunk_slots"]
    nops = sched["nops"]

    tslots_lo = sum(chunk_slots["lo"])
    tslots_hi = sum(chunk_slots["hi"])
    max_slots = {s: max(chunk_slots[s]) for s in ("lo", "hi")}
    max_ops_chunk = max(
        int((K["lo"] + K["hi"])[w0:w1].sum()) for (w0, w1) in CHUNKS)

    nc = bacc.Bacc("TRN2", target_bir_lowering=False, debug=False,
                   num_devices=NCORES, num_swdge_queues=N_QUEUES)
    qctr = [0]  # round-robin gather queue: queue q runs on Q7 cores 2q,2q+1

    t_featT = nc.dram_tensor("featT", [D_IN, SHARD_PAD], BF16, kind="ExternalInput")
    t_w1 = nc.dram_tensor("w1", [D_IN, D_H], BF16, kind="ExternalInput")
    t_w2 = nc.dram_tensor("w2", [D_H, D_Y2], BF16, kind="ExternalInput")
    t_idx = {s: nc.dram_tensor(f"idx_{s}", [128, max(8 * tslots_lo if s == "lo" else 8 * tslots_hi, 8)],
                               I16, kind="ExternalInput") for s in ("lo", "hi")}
    t_oh = nc.dram_tensor("oh", [128, nops * WIN], BF16, kind="ExternalInput")
    t_out = nc.dram_tensor("out", [SHARD_PAD, D_O], F32, kind="ExternalOutput")
    t_b1 = (nc.dram_tensor("b1rep", [128, D_H], F32, kind="ExternalInput")
            if with_b1 else None)

    with tile.TileContext(nc) as tc:
        with tc.tile_pool(name="dram", bufs=1, space="DRAM") as dram:
            x1_mine = dram.tile([SHARD, D_H], X1DT)
            x2_mine = dram.tile([SHARD_PAD, D_H], BF16)
            y2_mine = dram.tile([SHARD, D_Y2], BF16)
            # Shared-scratchpad outputs: the AllGather writes each core's
            # shard once into one chip-shared buffer instead of 8 local
            # replicas (the NRT fast path the HBM-HBM warning points at).
            x1_full = nc.dram_tensor(
                "x1_full_sh", [N_NODES, D_H], X1DT,
                kind="Internal", addr_space="Shared").ap()
            y2_full = nc.dram_tensor(
                "y2_full_sh", [N_NODES, D_Y2], BF16,
                kind="Internal", addr_space="Shared").ap()

            # ---------------- stage 0: X1_mine = feat @ W1 ----------------
            with (
                tc.tile_pool(name="s0w", bufs=1) as s0w,
                tc.tile_pool(name="s0f", bufs=1) as s0f,
                tc.tile_pool(name="s0d", bufs=4) as s0d,
                tc.tile_pool(name="s0p", bufs=6, space="PSUM") as s0p,
            ):
                w1_t = s0w.tile([128, 6, D_H], BF16)
                nc.sync.dma_start(
                    w1_t[:], t_w1.ap().rearrange("(k p) e -> p k e", p=128))
                ft = s0f.tile([128, 6, SHARD_PAD], BF16)
                ftv = t_featT.ap().rearrange("(k p) e -> p k e", p=128)
                for k in range(6):
                    nc.sync.dma_start(ft[:, k, :], ftv[:, k, :])
                # SUBAG: sub-AG k fires as soon as ranks < SUB_BOUNDS[k+1]
                # are written, overlapping the remaining stage-0 tiles.
                ag1_at = {SUB_BOUNDS[k + 1] - 1: k for k in range(3)}
                ag1_at[SHARD - 1] = 3
                for r in range(NRT):
                    ps = s0p.tile([128, D_H], F32)
                    for k in range(6):
                        nc.tensor.matmul(
                            ps[:], ft[:, k, r * 128:(r + 1) * 128], w1_t[:, k, :],
                            start=(k == 0), stop=(k == 5))
                    stg = s0d.tile([128, D_H], X1DT)
                    nc.vector.tensor_copy(stg[:], ps[:])
                    nrows = min(128, SHARD - r * 128)
                    nc.sync.dma_start(
                        x1_mine[r * 128:r * 128 + nrows, :], stg[0:nrows, :])
                    k_ag = ag1_at.get(r * 128 + nrows - 1)
                    if SUBAG and k_ag is not None and stage >= 2:
                        s0, s1 = SUB_BOUNDS[k_ag], SUB_BOUNDS[k_ag + 1]
                        nc.gpsimd.collective_compute(
                            "AllGather", mybir.AluOpType.bypass,
                            replica_groups=[list(range(NCORES))],
                            ins=[x1_mine[s0:s1, :].opt()],
                            outs=[x1_full[8 * s0:8 * s1, :].opt()])
                if not SUBAG and stage >= 2:
                    nc.gpsimd.collective_compute(
                        "AllGather", mybir.AluOpType.bypass,
                        replica_groups=[list(range(NCORES))],
                        ins=[x1_mine.opt()], outs=[x1_full.opt()])

            # ---------------- SpMM over a table ----------------
            def spmm(table_ap, elem, rhs_cols, drain, compact, label,
                     post_window=None, msg_dt=BF16):
                """Gather+one-hot-matmul all windows of one layer."""
                with (
                    tc.tile_pool(name=f"ms_lo_{label}", bufs=2) as mp_lo,
                    tc.tile_pool(name=f"ms_hi_{label}", bufs=2) as mp_hi,
                    tc.tile_pool(name=f"ohp_{label}", bufs=3) as ohp,
                    tc.tile_pool(name=f"ixp_{label}", bufs=2) as ixp,
                    tc.tile_pool(name=f"wps_{label}", bufs=6, space="PSUM") as wps,
                ):
                    mp = {"lo": mp_lo, "hi": mp_hi}
                    tab = {"lo": table_ap[0:LO_ROWS, :],
                           "hi": table_ap[LO_ROWS:N_NODES, :]}
                    slot_off = {"lo": 0, "hi": 0}
                    op_off = 0
                    for c, (cw0, cw1) in enumerate(CHUNKS):
                        msgs, nsl = {}, {}
                        for s in ("lo", "hi"):
                            n_slots = chunk_slots[s][c]
                            nsl[s] = n_slots
                            if n_slots == 0:
                                continue
                            it = ixp.tile([128, 8 * max_slots[s]], I16,
                                          tag=f"ix{s}")
                            nc.sync.dma_start(
                                it[:, 0:8 * n_slots],
                                t_idx[s].ap()[:, 8 * slot_off[s]:
                                              8 * (slot_off[s] + n_slots)])
                            mt = mp[s].tile([128, max_slots[s], elem], msg_dt,
                                            tag=f"m{s}")
                            # Split each stream's gather across both SWDGE
                            # queues (queue q's desc-gen runs on Q7 cores
                            # 2q,2q+1) so both core pairs stay busy.
                            n0 = (n_slots + 1) // 2
                            qa = qctr[0] % N_QUEUES
                            qb = (qctr[0] + 1) % N_QUEUES
                            qctr[0] += 1
                            for a, b, qn in ((0, n0, qa), (n0, n_slots, qb)):
                                if b > a:
                                    nc.gpsimd.dma_gather(
                                        mt[:, a:b, :], tab[s],
                                        it[:, 8 * a:8 * b],
                                        (b - a) * 128, (b - a) * 128, elem,
                                        single_packet=SINGLE_PACKET,
                                        queue_num=qn)
                            msgs[s] = mt
                        n_ops_c = int((K["lo"] + K["hi"])[cw0:cw1].sum())
                        oht = ohp.tile([128, max_ops_chunk * WIN], BF16, tag="oh")
                        nc.sync.dma_start(
                            oht[:, 0:n_ops_c * WIN],
                            t_oh.ap()[:, op_off * WIN:(op_off + n_ops_c) * WIN])
                        # matmuls
                        oc = 0
                        sl = {"lo": 0, "hi": 0}
                        for w in range(cw0, cw1):
                            n_ops_w = int(K["lo"][w] + K["hi"][w])
                            ps = wps.tile([WIN, rhs_cols], F32, tag="win")
                            done = 0
                            for s in ("lo", "hi"):
                                for j in range(int(K[s][w])):
                                    nc.tensor.matmul(
                                        ps[:],
                                        oht[:, (oc + done) * WIN:(oc + done + 1) * WIN],
                                        msgs[s][:, sl[s] + j, 0:rhs_cols],
                                        start=(done == 0),
                                        stop=(done == n_ops_w - 1))
                                    done += 1
                                sl[s] += int(K[s][w])
                            oc += n_ops_w
                            drain(w, ps, compact)
                            if post_window is not None:
                                post_window(w)
                        for s in ("lo", "hi"):
                            slot_off[s] += nsl[s]
                        op_off += n_ops_c

            # ------- SpMM-1: relu drains -> X2; Y2 fused per window-pair ----
            if stage >= 3:
              with (
                tc.tile_pool(name="cmp1", bufs=1) as cmp1,
                tc.tile_pool(name="y2w", bufs=1) as y2w,
                tc.tile_pool(name="y2t", bufs=1) as y2t,
                tc.tile_pool(name="y2d", bufs=4) as y2d,
                tc.tile_pool(name="y2p", bufs=2, space="PSUM") as y2p,
              ):
                compact1 = cmp1.tile([128, NRT, D_H], BF16)
                if with_b1:
                    b1_sb = cmp1.tile([128, D_H], F32)
                    nc.sync.dma_start(b1_sb[:], t_b1.ap())
                w2_t = y2w.tile([128, 2, D_Y2], BF16)
                nc.sync.dma_start(
                    w2_t[:], t_w2.ap().rearrange("(k p) e -> p k e", p=128))
                x2T = y2t.tile([128, 2, SHARD_PAD], BF16)

                def drain1(w, ps, compact):
                    p0 = (w % 2) * 64
                    dst_sl = compact[p0:p0 + 64, w // 2, :]
                    if with_b1:
                        nc.vector.tensor_add(dst_sl, ps[:], b1_sb[p0:p0 + 64, :])
                        nc.scalar.activation(
                            dst_sl, dst_sl, mybir.ActivationFunctionType.Relu)
                    else:
                        nc.scalar.activation(
                            dst_sl, ps[:], mybir.ActivationFunctionType.Relu)

                def emit_y2_mm(a):
                    """@W2 for a transposed block -> y2_mine rows."""
                    ps2 = y2p.tile([128, D_Y2], F32, tag="psy", name=f"psy{a}")
                    for k in range(2):
                        nc.tensor.matmul(
                            ps2[:], x2T[:, k, a * 128:(a + 1) * 128],
                            w2_t[:, k, :], start=(k == 0), stop=(k == 1))
                    stg2 = y2d.tile([128, D_Y2], BF16, tag="stg2", name=f"sg{a}")
                    nc.vector.tensor_copy(stg2[:], ps2[:])
                    nrows = min(128, SHARD - a * 128)
                    nc.sync.dma_start(
                        y2_mine[a * 128:a * 128 + nrows, :], stg2[0:nrows, :])
                    if SUBAG:
                        k_ag = {SUB_BOUNDS[k + 1] // 128 - 1: k
                                for k in range(3)}
                        k_ag[NRT - 1] = 3
                        k2 = k_ag.get(a)
                        if k2 is not None:
                            s0, s1 = SUB_BOUNDS[k2], SUB_BOUNDS[k2 + 1]
                            nc.gpsimd.collective_compute(
                                "AllGather", mybir.AluOpType.bypass,
                                replica_groups=[list(range(NCORES))],
                                ins=[y2_mine[s0:s1, :].opt()],
                                outs=[y2_full[8 * s0:8 * s1, :].opt()])

                def post_window1(w):
                    # after the odd window of a 128-row block: stream the
                    # block through X2 -> transpose; the @W2 matmuls for the
                    # PREVIOUS block are emitted here so the in-order PE
                    # never waits head-of-line on a transpose in flight
                    if stage < 4 or w % 2 == 0:
                        return
                    a = w // 2
                    nc.sync.dma_start(
                        x2_mine[a * 128:(a + 1) * 128, :], compact1[:, a, :])
                    for k in range(2):
                        nc.sync.dma_start(
                            x2T[:, k, a * 128:(a + 1) * 128],
                            x2_mine[a * 128:(a + 1) * 128,
                                    k * 128:(k + 1) * 128],
                            transpose=True)
                    if a > 0:
                        emit_y2_mm(a - 1)

                spmm(x1_full[:], D_H, D_H, drain1, compact1, "l1",
                     post_window=post_window1, msg_dt=X1DT)
                if stage >= 4:
                    emit_y2_mm(NRT - 1)

            if not SUBAG and stage >= 4:
                nc.gpsimd.collective_compute(
                    "AllGather", mybir.AluOpType.bypass,
                    replica_groups=[list(range(NCORES))],
                    ins=[y2_mine.opt()], outs=[y2_full.opt()])

            # ---------------- SpMM-2: copy drains -> out ----------------
            if stage >= 5:
              with tc.tile_pool(name="cmp2", bufs=1) as cmp2:
                compact2 = cmp2.tile([128, NRT, D_O], F32)

                def drain2(w, ps, compact):
                    p0 = (w % 2) * 64
                    nc.vector.tensor_copy(
                        compact[p0:p0 + 64, w // 2, :], ps[:])

                outv = t_out.ap().rearrange("(a p) e -> p a e", p=128)

                def post_window2(w):
                    # stream each finished 128-row block to HBM so the final
                    # store is hidden under the remaining L2 gathers
                    if w % 2 == 1:
                        a = w // 2
                        nc.sync.dma_start(outv[:, a, :], compact2[:, a, :])

                spmm(y2_full[:], D_Y2, D_O, drain2, compact2, "l2",
                     post_window=post_window2)

    nc.compile()
    return nc


# ----------------------------------------------------------------------------
# Entry point
# ----------------------------------------------------------------------------

_CACHE = {}


def _prepare(feature, src, dst, edge_w, W1, b1, W2):
    sched, payloads = _build_host_data(src, dst, edge_w)
    with_b1 = bool(np.any(np.asarray(b1) != 0))
    nc = _build_program(sched, with_b1=with_b1)

    W1p = np.zeros((D_IN, D_H), np.float32)
    W1p[:, :W1.shape[1]] = np.asarray(W1, np.float32)
    W2p = np.zeros((D_H, D_Y2), np.float32)
    W2p[:W2.shape[0], :W2.shape[1]] = np.asarray(W2, np.float32)
    feat = np.asarray(feature, np.float32)
    core_of, rank_of = sched["core_of"], sched["rank_of"]

    in_maps = []
    for m in range(NCORES):
        nodes = np.where(core_of == m)[0]
        nodes = nodes[np.argsort(rank_of[nodes])]
        fshard = np.zeros((SHARD_PAD, D_IN), np.float32)
        fshard[:SHARD] = feat[nodes]
        im = {
            "featT": np.ascontiguousarray(fshard.T).astype(nbf16),
            "w1": W1p.astype(nbf16),
            "w2": W2p.astype(nbf16),
            "oh": payloads[m]["oh"],
        }
        for s in ("lo", "hi"):
            arr = payloads[m]["idx_" + s]
            want = max(arr.shape[1], 8)
            buf = np.zeros((128, want), np.int16)
            buf[:, :arr.shape[1]] = arr
            im["idx_" + s] = buf
        if with_b1:
            b1p = np.zeros(D_H, np.float32)
            b1p[:np.asarray(b1).shape[0]] = np.asarray(b1, np.float32)
            im["b1rep"] = np.tile(b1p[None, :], (128, 1))
        in_maps.append(im)
    return nc, in_maps, sched


def kernel(feature, src, dst, edge_w, W1, b1, W2, b2, _trace=False):
    from concourse import bass_utils

    nc, in_maps, sched = _prepare(feature, src, dst, edge_w, W1, b1, W2)
    res = bass_utils.run_bass_kernel_spmd(
        nc, in_maps, core_ids=list(range(NCORES)), trace=_trace)

    d_out = W2.shape[1]
    out = np.empty((N_NODES, d_out), np.float32)
    core_of, rank_of = sched["core_of"], sched["rank_of"]
    for m in range(NCORES):
        nodes = np.where(core_of == m)[0]
        out[nodes] = res.results[m]["out"][rank_of[nodes], :d_out]
    out = out + np.asarray(b2, np.float32)[None, :]
    if _trace:
        kernel.last_exec_time_ns = res.exec_time_ns
    return out.astype(np.float32)

